# revision 20
# baseline (speedup 1.0000x reference)
"""Causal multi-head attention block (QKV proj + flash-style attention + out proj)
for Trainium2, sharded over 8 NeuronCores as (batch, head-group):
core c -> batch b = c//2, heads hg*4..hg*4+4 with hg = c%2.

Each core computes, for its batch and its 4 heads:
  QKV projection (bf16 matmuls, fp32 PSUM)
  S^T = K @ Q^T per (128k x 512q) tile, causal-pruned
  P = exp(SCALE * S^T)  (no max subtraction: scores are O(1) by construction)
  O^T = V^T-chunks @ P  accumulated over k-tiles, l = ones^T @ P (row sums)
  O^T_norm = O^T * broadcast(1/l)
  partial out = sum_h O_h^T.T @ Wproj_h (+ bias on even cores)
Host sums the two per-batch partials to unshard.
"""

import numpy as np
import ml_dtypes

import concourse.bass as bass
import concourse.tile as tile
from concourse import bacc, mybir
from concourse.bass_utils import run_bass_kernel_spmd

B, N, C, H = 4, 2048, 256, 8
SCALE = C ** -0.5
BF16 = ml_dtypes.bfloat16
FP32 = mybir.dt.float32
BF = mybir.dt.bfloat16
HPC = 4  # heads per core


def _emit(tc, nq, aps):
    nc = tc.nc
    nb = nq // 512   # 512-wide n/q blocks
    nt = nq // 128   # 128-wide n chunks

    xt_d, wqkv_d, wproj_d, bias_d, mask_d, ones_d, out_d = aps
    xt_r = xt_d.rearrange("(c p) n -> p c n", p=128)
    wqkv_r = wqkv_d.rearrange("(c p) m -> p c m", p=128)
    wproj_r = wproj_d.rearrange("(t p) f -> p t f", p=128)
    out_r = out_d.rearrange("(t p) f -> p t f", p=128)

    singles = tc._es.enter_context(tc.tile_pool(name="singles", bufs=1))
    pool_qkv = tc._es.enter_context(tc.tile_pool(name="qkvp", bufs=2))
    pool_p = tc._es.enter_context(tc.tile_pool(name="pp", bufs=6))
    pool_misc = tc._es.enter_context(tc.tile_pool(name="miscp", bufs=2))
    pool_osb = tc._es.enter_context(tc.tile_pool(name="osbp", bufs=6))
    pool_ot = tc._es.enter_context(tc.tile_pool(name="psumot", bufs=2, space="PSUM"))
    pool_s = tc._es.enter_context(tc.tile_pool(name="psums", bufs=3, space="PSUM"))
    pool_l = tc._es.enter_context(tc.tile_pool(name="psuml", bufs=1, space="PSUM"))

    # --- load constants / inputs ---
    xt_sb = singles.tile([128, 2, nq], BF)
    wqkv_sb = singles.tile([128, 2, 3 * HPC * C], BF)
    wproj_sb = singles.tile([128, 2 * HPC, C], BF)
    bias_sb = singles.tile([1, C], FP32)
    biasb_sb = singles.tile([128, C], FP32)
    mask_sb = singles.tile([128, 128], BF)
    ones_sb = singles.tile([128, 1], BF)
    ot_sb = singles.tile([128, 2 * HPC, nq], BF)

    # split the critical input DMAs into chunks, spread across BOTH HW-DGE
    # rings (sync + scalar issue different rings) so the first QKV matmuls
    # can start as early as possible
    for ib in range(nb):
        nc.sync.dma_start(xt_sb[:, :, ib * 512:(ib + 1) * 512],
                          xt_r[:, :, ib * 512:(ib + 1) * 512])
    for hp in range(HPC):
        c0 = hp * 3 * C
        nc.scalar.dma_start(wqkv_sb[:, :, c0:c0 + 3 * C], wqkv_r[:, :, c0:c0 + 3 * C])
    nc.sync.dma_start(mask_sb[:], mask_d[:])
    nc.sync.dma_start(ones_sb[:], ones_d[:])
    nc.scalar.dma_start(wproj_sb[:], wproj_r)
    nc.scalar.dma_start(bias_sb[:], bias_d[:])
    nc.gpsimd.partition_broadcast(biasb_sb[:], bias_sb[:])

    # warm up the PE HAM clock gate with dummy matmuls while input DMAs land
    warm_sb = singles.tile([128, 512], BF)
    nc.gpsimd.memset(warm_sb[:], 0.0)
    warm_ps = pool_ot.tile([128, 512], FP32, tag="ot0", name="warm_ps")
    for wi in range(30):
        nc.tensor.matmul(warm_ps[:], warm_sb[:, :128], warm_sb[:],
                         start=(wi == 0), stop=(wi == 29))

    for hp in range(HPC):
        # --- QKV projection for this head ---
        qt_sb = pool_qkv.tile([128, 2, nq], BF, tag="qt", name="qt")
        kt_sb = pool_qkv.tile([128, 2, nq], BF, tag="kt", name="kt")
        v_sb = pool_qkv.tile([128, nt, C], BF, tag="v", name="v")
        qkv_pools = [(pool_s, "s"), (pool_ot, "ot0"), (pool_ot, "ot1")]
        blk = 0
        for j, tgt in ((0, qt_sb), (1, kt_sb)):
            for ct in range(2):
                col0 = (hp * 3 + j) * C + ct * 128
                for ib in range(nb):
                    pp, ptag = qkv_pools[blk % 3]
                    ps = pp.tile([128, 512], FP32, tag=ptag, name="ps_qk")
                    for ci in range(2):
                        nc.tensor.matmul(
                            ps[:],
                            wqkv_sb[:, ci, col0:col0 + 128],
                            xt_sb[:, ci, ib * 512:(ib + 1) * 512],
                            start=(ci == 0), stop=(ci == 1),
                        )
                    if blk % 2 == 0:
                        nc.scalar.copy(tgt[:, ct, ib * 512:(ib + 1) * 512], ps[:])
                    else:
                        nc.vector.tensor_copy(tgt[:, ct, ib * 512:(ib + 1) * 512], ps[:])
                    blk += 1
        vcol = (hp * 3 + 2) * C
        for it in range(nt):
            pp, ptag = qkv_pools[blk % 3]
            ps = pp.tile([128, 512], FP32, tag=ptag, name="ps_v")
            for ci in range(2):
                nc.tensor.matmul(
                    ps[:, :C],
                    xt_sb[:, ci, it * 128:(it + 1) * 128],
                    wqkv_sb[:, ci, vcol:vcol + C],
                    start=(ci == 0), stop=(ci == 1),
                )
            if blk % 2 == 0:
                nc.scalar.copy(v_sb[:, it, :], ps[:, :C])
            else:
                nc.vector.tensor_copy(v_sb[:, it, :], ps[:, :C])
            blk += 1

        # --- flash attention over this head, causal ---
        steps = []
        for qb in range(nb):
            kmax = 4 * qb + 4
            for kt in range(kmax):
                q_off = max(0, kt * 128 - qb * 512)
                steps.append((qb, kt, q_off, 512 - q_off, kt == 0, kt == kmax - 1))

        state = {}
        s_ring = [pool_s.tile([128, 512], FP32, tag="s", name=f"sring{i}")
                  for i in range(3)]
        p_ring = [pool_p.tile([128, 512], BF, tag="p", name=f"pring{i}")
                  for i in range(6)]
        ring = {"i": 0}

        def emit_S(step):
            qb, kt, q_off, nqf, first, last = step
            if first:
                state[qb] = (
                    pool_ot.tile([128, 512], FP32, tag="ot0", name="ot0"),
                    pool_ot.tile([128, 512], FP32, tag="ot1", name="ot1"),
                    pool_l.tile([1, 512], FP32, tag="l", name="lp"),
                )
            q0 = qb * 512 + q_off
            ri = ring["i"]; ring["i"] += 1
            s_ps = s_ring[ri % 3]
            for ci in range(2):
                nc.tensor.matmul(
                    s_ps[:, :nqf],
                    kt_sb[:, ci, kt * 128:(kt + 1) * 128],
                    qt_sb[:, ci, q0:q0 + nqf],
                    start=(ci == 0), stop=(ci == 1),
                )
            p_sb = p_ring[ri % 6]
            nc.scalar.activation(
                p_sb[:, :nqf], s_ps[:, :nqf],
                mybir.ActivationFunctionType.Exp, scale=SCALE,
            )
            if kt >= 4 * qb:  # diagonal tile: causal mask on first 128 cols
                nc.vector.tensor_tensor(
                    p_sb[:, :128], p_sb[:, :128], mask_sb[:], mybir.AluOpType.mult
                )
            return p_sb

        def emit_PV(step, p_sb):
            qb, kt, q_off, nqf, first, last = step
            ot0, ot1, lp = state[qb]
            nc.tensor.matmul(ot1[:, q_off:], v_sb[:, kt, 0:128], p_sb[:, :nqf],
                             start=first, stop=last)
            nc.tensor.matmul(ot0[:, q_off:], v_sb[:, kt, 128:256], p_sb[:, :nqf],
                             start=first, stop=last)

        def emit_L(step, p_sb):
            qb, kt, q_off, nqf, first, last = step
            ot0, ot1, lp = state[qb]
            nc.tensor.matmul(lp[:, q_off:], ones_sb[:], p_sb[:, :nqf],
                             start=first, stop=last)
            if last:
                rl_sb = pool_misc.tile([1, 512], FP32, tag="rl", name="rl")
                rb_sb = pool_misc.tile([128, 512], FP32, tag="rb", name="rb")
                nc.vector.reciprocal_approx_fast(rl_sb[:], lp[:])
                nc.gpsimd.partition_broadcast(rb_sb[:], rl_sb[:])
                for ct, otp in ((0, ot1), (1, ot0)):
                    nc.vector.tensor_tensor(
                        ot_sb[:, hp * 2 + ct, qb * 512:(qb + 1) * 512],
                        otp[:], rb_sb[:], mybir.AluOpType.mult,
                    )

        # software pipeline; PE emission order per slot is
        #   S(i), L(i-2), PV(i-1)
        # so exp(i-1) on ACT is covered by S(i)+L(i-2) of PE work
        work = []  # parallel to steps: (step, p_sb)
        for i, step in enumerate(steps):
            work.append((step, emit_S(step)))
            if i >= 2:
                emit_L(*work[i - 2])
            if i >= 1:
                emit_PV(*work[i - 1])
        n = len(steps)
        if n >= 2:
            emit_L(*work[n - 2])
        emit_PV(*work[n - 1])
        emit_L(*work[n - 1])

    # --- output projection, all heads accumulated in PSUM ---
    # round-robin psum across all three tag families (attention pools are
    # done by now) for more outstanding chunks
    prj_pools = [(pool_s, "s"), (pool_ot, "ot0"), (pool_ot, "ot1")]
    for it in range(nt):
        pp, ptag = prj_pools[it % 3]
        ps = pp.tile([128, 512], FP32, tag=ptag, name="ps_prj")
        for t in range(2 * HPC):
            nc.tensor.matmul(
                ps[:, :C],
                ot_sb[:, t, it * 128:(it + 1) * 128],
                wproj_sb[:, t, :],
                start=(t == 0), stop=(t == 2 * HPC - 1),
            )
        osb = pool_osb.tile([128, C], FP32, tag="osb", name="osb")
        nc.vector.tensor_tensor(osb[:], ps[:, :C], biasb_sb[:], mybir.AluOpType.add)
        nc.sync.dma_start(out_r[:, it, :], osb[:])


def build_program(nq=N):
    nc = bacc.Bacc(trn_type="TRN2")
    xt_d = nc.dram_tensor("xt", (C, nq), BF, kind="ExternalInput").ap()
    wqkv_d = nc.dram_tensor("wqkv", (C, 3 * HPC * C), BF, kind="ExternalInput").ap()
    wproj_d = nc.dram_tensor("wproj", (2 * HPC * 128, C), BF, kind="ExternalInput").ap()
    bias_d = nc.dram_tensor("bias", (1, C), FP32, kind="ExternalInput").ap()
    mask_d = nc.dram_tensor("mask", (128, 128), BF, kind="ExternalInput").ap()
    ones_d = nc.dram_tensor("ones", (128, 1), BF, kind="ExternalInput").ap()
    out_d = nc.dram_tensor("out", (nq, C), FP32, kind="ExternalOutput").ap()
    with tile.TileContext(nc) as tc:
        import contextlib
        tc._es = contextlib.ExitStack()
        with tc._es:
            _emit(tc, nq, (xt_d, wqkv_d, wproj_d, bias_d, mask_d, ones_d, out_d))
    nc.compile()
    return nc


def core_inputs(core, x, w_qkv, w_proj, b_proj, nq=N):
    b, hg = core // 2, core % 2
    heads = list(range(hg * HPC, hg * HPC + HPC))
    xt = np.ascontiguousarray(x[b].T).astype(BF16)
    wr = np.asarray(w_qkv, np.float32).reshape(C, 3, H, C)
    w4 = np.ascontiguousarray(
        wr[:, :, heads, :].transpose(0, 2, 1, 3)
    ).reshape(C, 3 * HPC * C).astype(BF16)
    wp = np.asarray(w_proj, np.float32).reshape(H, C, C)[heads].reshape(HPC * C, C).astype(BF16)
    bias = (np.asarray(b_proj, np.float32) if hg == 0
            else np.zeros(C, np.float32)).reshape(1, C)
    mask = (np.arange(128)[:, None] <= np.arange(128)[None, :]).astype(BF16)
    ones = np.ones((128, 1), BF16)
    return {"xt": xt, "wqkv": w4, "wproj": wp, "bias": bias,
            "mask": mask, "ones": ones}


_CACHE = {}


def kernel(x, w_qkv, w_proj, b_proj, **run_kwargs):
    if "nc" not in _CACHE:
        _CACHE["nc"] = build_program(N)
    nc = _CACHE["nc"]
    in_maps = [core_inputs(c, x, w_qkv, w_proj, b_proj) for c in range(8)]
    res = run_bass_kernel_spmd(nc, in_maps, core_ids=list(range(8)), **run_kwargs)
    out = np.zeros((B, N, C), np.float32)
    for c in range(8):
        out[c // 2] += res.results[c]["out"]
    _CACHE["last_results"] = res
    return out


# revision 22
# speedup vs baseline: 1.0034x; 1.0034x over previous
"""Causal multi-head attention block (QKV proj + flash-style attention + out proj)
for Trainium2, sharded over 8 NeuronCores as (batch, head-group):
core c -> batch b = c//2, heads hg*4..hg*4+4 with hg = c%2.

Each core computes, for its batch and its 4 heads:
  QKV projection (bf16 matmuls, fp32 PSUM)
  S^T = K @ Q^T per (128k x 512q) tile, causal-pruned
  P = exp(SCALE * S^T)  (no max subtraction: scores are O(1) by construction)
  O^T = V^T-chunks @ P  accumulated over k-tiles, l = ones^T @ P (row sums)
  O^T_norm = O^T * broadcast(1/l)
  partial out = sum_h O_h^T.T @ Wproj_h (+ bias on even cores)
Host sums the two per-batch partials to unshard.
"""

import numpy as np
import ml_dtypes

import concourse.bass as bass
import concourse.tile as tile
from concourse import bacc, mybir
from concourse.bass_utils import run_bass_kernel_spmd

B, N, C, H = 4, 2048, 256, 8
SCALE = C ** -0.5
BF16 = ml_dtypes.bfloat16
FP32 = mybir.dt.float32
BF = mybir.dt.bfloat16
HPC = 4  # heads per core


def _emit(tc, nq, aps):
    nc = tc.nc
    nb = nq // 512   # 512-wide n/q blocks
    nt = nq // 128   # 128-wide n chunks

    xt_d, wqkv_d, wproj_d, bias_d, mask_d, ones_d, out_d = aps
    xt_r = xt_d.rearrange("(c p) n -> p c n", p=128)
    wqkv_r = wqkv_d.rearrange("(c p) m -> p c m", p=128)
    wproj_r = wproj_d.rearrange("(t p) f -> p t f", p=128)
    out_r = out_d.rearrange("(t p) f -> p t f", p=128)

    singles = tc._es.enter_context(tc.tile_pool(name="singles", bufs=1))
    pool_qkv = tc._es.enter_context(tc.tile_pool(name="qkvp", bufs=2))
    pool_p = tc._es.enter_context(tc.tile_pool(name="pp", bufs=6))
    pool_misc = tc._es.enter_context(tc.tile_pool(name="miscp", bufs=2))
    pool_osb = tc._es.enter_context(tc.tile_pool(name="osbp", bufs=6))
    pool_ot = tc._es.enter_context(tc.tile_pool(name="psumot", bufs=2, space="PSUM"))
    pool_s = tc._es.enter_context(tc.tile_pool(name="psums", bufs=3, space="PSUM"))
    pool_l = tc._es.enter_context(tc.tile_pool(name="psuml", bufs=1, space="PSUM"))

    # --- load constants / inputs ---
    xt_sb = singles.tile([128, 2, nq], BF)
    wqkv_sb = singles.tile([128, 2, 3 * HPC * C], BF)
    wproj_sb = singles.tile([128, 2 * HPC, C], BF)
    bias_sb = singles.tile([1, C], FP32)
    biasb_sb = singles.tile([128, C], FP32)
    mask_sb = singles.tile([128, 128], BF)
    ones_sb = singles.tile([128, 1], BF)
    ot_sb = singles.tile([128, 2 * HPC, nq], BF)

    # split the critical input DMAs into chunks, spread across BOTH HW-DGE
    # rings (sync + scalar issue different rings) so the first QKV matmuls
    # can start as early as possible
    for ib in range(nb):
        nc.sync.dma_start(xt_sb[:, :, ib * 512:(ib + 1) * 512],
                          xt_r[:, :, ib * 512:(ib + 1) * 512])
    for hw in range(HPC):
        c0 = hw * 3 * C
        nc.scalar.dma_start(wqkv_sb[:, :, c0:c0 + 3 * C], wqkv_r[:, :, c0:c0 + 3 * C])
    nc.sync.dma_start(mask_sb[:], mask_d[:])
    nc.sync.dma_start(ones_sb[:], ones_d[:])
    nc.scalar.dma_start(wproj_sb[:], wproj_r)
    nc.scalar.dma_start(bias_sb[:], bias_d[:])
    nc.gpsimd.partition_broadcast(biasb_sb[:], bias_sb[:])

    # warm up the PE HAM clock gate with dummy matmuls while input DMAs land
    warm_sb = singles.tile([128, 512], BF)
    nc.gpsimd.memset(warm_sb[:], 0.0)
    warm_ps = pool_ot.tile([128, 512], FP32, tag="ot0", name="warm_ps")
    for wi in range(30):
        nc.tensor.matmul(warm_ps[:], warm_sb[:, :128], warm_sb[:],
                         start=(wi == 0), stop=(wi == 29))

    s_ring = [pool_s.tile([128, 512], FP32, tag="s", name=f"sring{i}")
              for i in range(3)]
    p_ring = [pool_p.tile([128, 512], BF, tag="p", name=f"pring{i}")
              for i in range(6)]
    ring = {"i": 0}

    def alloc_head_tiles():
        qt_sb = pool_qkv.tile([128, 2, nq], BF, tag="qt", name="qt")
        kt_sb = pool_qkv.tile([128, 2, nq], BF, tag="kt", name="kt")
        v_sb = pool_qkv.tile([128, nt, C], BF, tag="v", name="v")
        return qt_sb, kt_sb, v_sb

    def qkv_blocks(hp, tiles):
        """One closure per (128,512) projection block of head hp."""
        qt_sb, kt_sb, v_sb = tiles
        blocks = []

        def qk_block(j, ct, ib, tgt, par):
            def go():
                ri = ring["i"]; ring["i"] += 1
                ps = s_ring[ri % 3]
                col0 = (hp * 3 + j) * C + ct * 128
                for ci in range(2):
                    nc.tensor.matmul(
                        ps[:],
                        wqkv_sb[:, ci, col0:col0 + 128],
                        xt_sb[:, ci, ib * 512:(ib + 1) * 512],
                        start=(ci == 0), stop=(ci == 1),
                    )
                if par % 2 == 0:
                    nc.scalar.copy(tgt[:, ct, ib * 512:(ib + 1) * 512], ps[:])
                else:
                    nc.vector.tensor_copy(tgt[:, ct, ib * 512:(ib + 1) * 512], ps[:])
            return go

        def v_block(it, par):
            def go():
                ri = ring["i"]; ring["i"] += 1
                ps = s_ring[ri % 3]
                vcol = (hp * 3 + 2) * C
                for ci in range(2):
                    nc.tensor.matmul(
                        ps[:, :C],
                        xt_sb[:, ci, it * 128:(it + 1) * 128],
                        wqkv_sb[:, ci, vcol:vcol + C],
                        start=(ci == 0), stop=(ci == 1),
                    )
                if par % 2 == 0:
                    nc.scalar.copy(v_sb[:, it, :], ps[:, :C])
                else:
                    nc.vector.tensor_copy(v_sb[:, it, :], ps[:, :C])
            return go

        par = 0
        for j, ti in ((0, 0), (1, 1)):
            for ct in range(2):
                for ib in range(nb):
                    blocks.append(qk_block(j, ct, ib, tiles[ti], par))
                    par += 1
        for it in range(nt):
            blocks.append(v_block(it, par))
            par += 1
        return blocks

    def attention(hp, tiles, next_blocks):
        """Flash attention for head hp; next head's QKV blocks are drip-fed
        into the PE stream to absorb per-cycle semaphore bubbles."""
        qt_sb, kt_sb, v_sb = tiles
        steps = []
        for qb in range(nb):
            kmax = 4 * qb + 4
            for kt in range(kmax):
                q_off = max(0, kt * 128 - qb * 512)
                steps.append((qb, kt, q_off, 512 - q_off, kt == 0, kt == kmax - 1))

        state = {}

        def emit_S(step):
            qb, kt, q_off, nqf, first, last = step
            if first:
                state[qb] = (
                    pool_ot.tile([128, 512], FP32, tag="ot0", name="ot0"),
                    pool_ot.tile([128, 512], FP32, tag="ot1", name="ot1"),
                    pool_l.tile([1, 512], FP32, tag="l", name="lp"),
                )
            q0 = qb * 512 + q_off
            ri = ring["i"]; ring["i"] += 1
            s_ps = s_ring[ri % 3]
            for ci in range(2):
                nc.tensor.matmul(
                    s_ps[:, :nqf],
                    kt_sb[:, ci, kt * 128:(kt + 1) * 128],
                    qt_sb[:, ci, q0:q0 + nqf],
                    start=(ci == 0), stop=(ci == 1),
                )
            p_sb = p_ring[ri % 6]
            nc.scalar.activation(
                p_sb[:, :nqf], s_ps[:, :nqf],
                mybir.ActivationFunctionType.Exp, scale=SCALE,
            )
            if kt >= 4 * qb:  # diagonal tile: causal mask on first 128 cols
                nc.vector.tensor_tensor(
                    p_sb[:, :128], p_sb[:, :128], mask_sb[:], mybir.AluOpType.mult
                )
            return p_sb

        def emit_PV(step, p_sb):
            qb, kt, q_off, nqf, first, last = step
            ot0, ot1, lp = state[qb]
            nc.tensor.matmul(ot1[:, q_off:], v_sb[:, kt, 0:128], p_sb[:, :nqf],
                             start=first, stop=last)
            nc.tensor.matmul(ot0[:, q_off:], v_sb[:, kt, 128:256], p_sb[:, :nqf],
                             start=first, stop=last)

        def emit_L(step, p_sb):
            qb, kt, q_off, nqf, first, last = step
            ot0, ot1, lp = state[qb]
            nc.tensor.matmul(lp[:, q_off:], ones_sb[:], p_sb[:, :nqf],
                             start=first, stop=last)
            if last:
                rl_sb = pool_misc.tile([1, 512], FP32, tag="rl", name="rl")
                rb_sb = pool_misc.tile([128, 512], FP32, tag="rb", name="rb")
                nc.vector.reciprocal_approx_fast(rl_sb[:], lp[:])
                nc.gpsimd.partition_broadcast(rb_sb[:], rl_sb[:])
                for ct, otp in ((0, ot1), (1, ot0)):
                    nc.vector.tensor_tensor(
                        ot_sb[:, hp * 2 + ct, qb * 512:(qb + 1) * 512],
                        otp[:], rb_sb[:], mybir.AluOpType.mult,
                    )

        # software pipeline; PE emission order per slot is
        #   S(i), L(i-2), PV(i-1), [next head's QKV block]
        work = []
        emitted = 0
        for i, step in enumerate(steps):
            work.append((step, emit_S(step)))
            if i >= 2:
                emit_L(*work[i - 2])
            if i >= 1:
                emit_PV(*work[i - 1])
            want = (i + 1) * len(next_blocks) // len(steps)
            while emitted < want:
                next_blocks[emitted]()
                emitted += 1
        n = len(steps)
        if n >= 2:
            emit_L(*work[n - 2])
        emit_PV(*work[n - 1])
        emit_L(*work[n - 1])
        while emitted < len(next_blocks):
            next_blocks[emitted]()
            emitted += 1

    head_tiles = alloc_head_tiles()
    for b in qkv_blocks(0, head_tiles):
        b()
    for hp in range(HPC):
        if hp + 1 < HPC:
            nxt_tiles = alloc_head_tiles()
            nxt = qkv_blocks(hp + 1, nxt_tiles)
        else:
            nxt_tiles, nxt = None, []
        attention(hp, head_tiles, nxt)
        head_tiles = nxt_tiles

    # --- output projection, all heads accumulated in PSUM ---
    # round-robin psum across all three tag families (attention pools are
    # done by now) for more outstanding chunks
    prj_pools = [(pool_s, "s"), (pool_ot, "ot0"), (pool_ot, "ot1")]
    for it in range(nt):
        pp, ptag = prj_pools[it % 3]
        ps = pp.tile([128, 512], FP32, tag=ptag, name="ps_prj")
        for t in range(2 * HPC):
            nc.tensor.matmul(
                ps[:, :C],
                ot_sb[:, t, it * 128:(it + 1) * 128],
                wproj_sb[:, t, :],
                start=(t == 0), stop=(t == 2 * HPC - 1),
            )
        osb = pool_osb.tile([128, C], FP32, tag="osb", name="osb")
        nc.vector.tensor_tensor(osb[:], ps[:, :C], biasb_sb[:], mybir.AluOpType.add)
        nc.sync.dma_start(out_r[:, it, :], osb[:])


def build_program(nq=N):
    nc = bacc.Bacc(trn_type="TRN2")
    xt_d = nc.dram_tensor("xt", (C, nq), BF, kind="ExternalInput").ap()
    wqkv_d = nc.dram_tensor("wqkv", (C, 3 * HPC * C), BF, kind="ExternalInput").ap()
    wproj_d = nc.dram_tensor("wproj", (2 * HPC * 128, C), BF, kind="ExternalInput").ap()
    bias_d = nc.dram_tensor("bias", (1, C), FP32, kind="ExternalInput").ap()
    mask_d = nc.dram_tensor("mask", (128, 128), BF, kind="ExternalInput").ap()
    ones_d = nc.dram_tensor("ones", (128, 1), BF, kind="ExternalInput").ap()
    out_d = nc.dram_tensor("out", (nq, C), FP32, kind="ExternalOutput").ap()
    with tile.TileContext(nc) as tc:
        import contextlib
        tc._es = contextlib.ExitStack()
        with tc._es:
            _emit(tc, nq, (xt_d, wqkv_d, wproj_d, bias_d, mask_d, ones_d, out_d))
    nc.compile()
    return nc


def core_inputs(core, x, w_qkv, w_proj, b_proj, nq=N):
    b, hg = core // 2, core % 2
    heads = list(range(hg * HPC, hg * HPC + HPC))
    xt = np.ascontiguousarray(x[b].T).astype(BF16)
    wr = np.asarray(w_qkv, np.float32).reshape(C, 3, H, C)
    w4 = np.ascontiguousarray(
        wr[:, :, heads, :].transpose(0, 2, 1, 3)
    ).reshape(C, 3 * HPC * C).astype(BF16)
    wp = np.asarray(w_proj, np.float32).reshape(H, C, C)[heads].reshape(HPC * C, C).astype(BF16)
    bias = (np.asarray(b_proj, np.float32) if hg == 0
            else np.zeros(C, np.float32)).reshape(1, C)
    mask = (np.arange(128)[:, None] <= np.arange(128)[None, :]).astype(BF16)
    ones = np.ones((128, 1), BF16)
    return {"xt": xt, "wqkv": w4, "wproj": wp, "bias": bias,
            "mask": mask, "ones": ones}


_CACHE = {}


def kernel(x, w_qkv, w_proj, b_proj, **run_kwargs):
    if "nc" not in _CACHE:
        _CACHE["nc"] = build_program(N)
    nc = _CACHE["nc"]
    in_maps = [core_inputs(c, x, w_qkv, w_proj, b_proj) for c in range(8)]
    res = run_bass_kernel_spmd(nc, in_maps, core_ids=list(range(8)), **run_kwargs)
    out = np.zeros((B, N, C), np.float32)
    for c in range(8):
        out[c // 2] += res.results[c]["out"]
    _CACHE["last_results"] = res
    return out


# revision 23
# speedup vs baseline: 1.0068x; 1.0034x over previous
"""Causal multi-head attention block (QKV proj + flash-style attention + out proj)
for Trainium2, sharded over 8 NeuronCores as (batch, head-group):
core c -> batch b = c//2, heads hg*4..hg*4+4 with hg = c%2.

Each core computes, for its batch and its 4 heads:
  QKV projection (bf16 matmuls, fp32 PSUM)
  S^T = K @ Q^T per (128k x 512q) tile, causal-pruned
  P = exp(SCALE * S^T)  (no max subtraction: scores are O(1) by construction)
  O^T = V^T-chunks @ P  accumulated over k-tiles, l = ones^T @ P (row sums)
  O^T_norm = O^T * broadcast(1/l)
  partial out = sum_h O_h^T.T @ Wproj_h (+ bias on even cores)
Host sums the two per-batch partials to unshard.
"""

import numpy as np
import ml_dtypes

import concourse.bass as bass
import concourse.tile as tile
from concourse import bacc, mybir
from concourse.bass_utils import run_bass_kernel_spmd

B, N, C, H = 4, 2048, 256, 8
SCALE = C ** -0.5
BF16 = ml_dtypes.bfloat16
FP32 = mybir.dt.float32
BF = mybir.dt.bfloat16
HPC = 4  # heads per core


def _emit(tc, nq, aps):
    nc = tc.nc
    nb = nq // 512   # 512-wide n/q blocks
    nt = nq // 128   # 128-wide n chunks

    xt_d, wqkv_d, wproj_d, bias_d, mask_d, ones_d, out_d = aps
    xt_r = xt_d.rearrange("(c p) n -> p c n", p=128)
    wqkv_r = wqkv_d.rearrange("(c p) m -> p c m", p=128)
    wproj_r = wproj_d.rearrange("(t p) f -> p t f", p=128)
    out_r = out_d.rearrange("(t p) f -> p t f", p=128)

    singles = tc._es.enter_context(tc.tile_pool(name="singles", bufs=1))
    pool_qkv = tc._es.enter_context(tc.tile_pool(name="qkvp", bufs=2))
    pool_p = tc._es.enter_context(tc.tile_pool(name="pp", bufs=6))
    pool_misc = tc._es.enter_context(tc.tile_pool(name="miscp", bufs=2))
    pool_osb = tc._es.enter_context(tc.tile_pool(name="osbp", bufs=6))
    pool_ot = tc._es.enter_context(tc.tile_pool(name="psumot", bufs=2, space="PSUM"))
    pool_s = tc._es.enter_context(tc.tile_pool(name="psums", bufs=3, space="PSUM"))
    pool_l = tc._es.enter_context(tc.tile_pool(name="psuml", bufs=1, space="PSUM"))

    # --- load constants / inputs ---
    xt_sb = singles.tile([128, 2, nq], BF)
    wqkv_sb = singles.tile([128, 2, 3 * HPC * C], BF)
    wproj_sb = singles.tile([128, 2 * HPC, C], BF)
    bias_sb = singles.tile([1, C], FP32)
    biasb_sb = singles.tile([128, C], FP32)
    mask_sb = singles.tile([128, 128], BF)
    ones_sb = singles.tile([128, 1], BF)
    ot_sb = singles.tile([128, 2 * HPC, nq], BF)

    # split the critical input DMAs into chunks, spread across BOTH HW-DGE
    # rings (sync + scalar issue different rings) so the first QKV matmuls
    # can start as early as possible
    for ib in range(nb):
        nc.sync.dma_start(xt_sb[:, :, ib * 512:(ib + 1) * 512],
                          xt_r[:, :, ib * 512:(ib + 1) * 512])
    for hw in range(HPC):
        c0 = hw * 3 * C
        nc.scalar.dma_start(wqkv_sb[:, :, c0:c0 + 3 * C], wqkv_r[:, :, c0:c0 + 3 * C])
    nc.sync.dma_start(mask_sb[:], mask_d[:])
    nc.sync.dma_start(ones_sb[:], ones_d[:])
    nc.scalar.dma_start(wproj_sb[:], wproj_r)
    nc.scalar.dma_start(bias_sb[:], bias_d[:])
    nc.gpsimd.partition_broadcast(biasb_sb[:], bias_sb[:])

    # warm up the PE HAM clock gate with dummy matmuls while input DMAs land
    warm_sb = singles.tile([128, 512], BF)
    nc.gpsimd.memset(warm_sb[:], 0.0)
    warm_ps = pool_ot.tile([128, 512], FP32, tag="ot0", name="warm_ps")
    for wi in range(30):
        nc.tensor.matmul(warm_ps[:], warm_sb[:, :128], warm_sb[:],
                         start=(wi == 0), stop=(wi == 29))

    s_ring = [pool_s.tile([128, 512], FP32, tag="s", name=f"sring{i}")
              for i in range(3)]
    p_ring = [pool_p.tile([128, 512], BF, tag="p", name=f"pring{i}")
              for i in range(6)]
    ring = {"i": 0}

    def alloc_head_tiles():
        qt_sb = pool_qkv.tile([128, 2, nq], BF, tag="qt", name="qt")
        kt_sb = pool_qkv.tile([128, 2, nq], BF, tag="kt", name="kt")
        v_sb = pool_qkv.tile([128, nt, C], BF, tag="v", name="v")
        return qt_sb, kt_sb, v_sb

    def qkv_blocks(hp, tiles):
        """One closure per (128,512) projection block of head hp."""
        qt_sb, kt_sb, v_sb = tiles
        blocks = []

        def qk_block(j, ct, ib, tgt, par):
            def go():
                ri = ring["i"]; ring["i"] += 1
                ps = s_ring[ri % 3]
                col0 = (hp * 3 + j) * C + ct * 128
                for ci in range(2):
                    nc.tensor.matmul(
                        ps[:],
                        wqkv_sb[:, ci, col0:col0 + 128],
                        xt_sb[:, ci, ib * 512:(ib + 1) * 512],
                        start=(ci == 0), stop=(ci == 1),
                    )
                if par % 2 == 0:
                    nc.scalar.copy(tgt[:, ct, ib * 512:(ib + 1) * 512], ps[:])
                else:
                    nc.vector.tensor_copy(tgt[:, ct, ib * 512:(ib + 1) * 512], ps[:])
            return go

        def v_block(it, par):
            def go():
                ri = ring["i"]; ring["i"] += 1
                ps = s_ring[ri % 3]
                vcol = (hp * 3 + 2) * C
                for ci in range(2):
                    nc.tensor.matmul(
                        ps[:, :C],
                        xt_sb[:, ci, it * 128:(it + 1) * 128],
                        wqkv_sb[:, ci, vcol:vcol + C],
                        start=(ci == 0), stop=(ci == 1),
                    )
                if par % 2 == 0:
                    nc.scalar.copy(v_sb[:, it, :], ps[:, :C])
                else:
                    nc.vector.tensor_copy(v_sb[:, it, :], ps[:, :C])
            return go

        par = 0
        for j, ti in ((0, 0), (1, 1)):
            for ct in range(2):
                for ib in range(nb):
                    blocks.append(qk_block(j, ct, ib, tiles[ti], par))
                    par += 1
        for it in range(nt):
            blocks.append(v_block(it, par))
            par += 1
        return blocks

    def attention(hp, tiles, next_blocks):
        """Flash attention for head hp; next head's QKV blocks are drip-fed
        into the PE stream to absorb per-cycle semaphore bubbles."""
        qt_sb, kt_sb, v_sb = tiles
        steps = []
        for qb in range(nb):
            kmax = 4 * qb + 4
            for kt in range(kmax):
                q_off = max(0, kt * 128 - qb * 512)
                steps.append((qb, kt, q_off, 512 - q_off, kt == 0, kt == kmax - 1))

        state = {}

        def emit_S(step):
            qb, kt, q_off, nqf, first, last = step
            if first:
                state[qb] = (
                    pool_ot.tile([128, 512], FP32, tag="ot0", name="ot0"),
                    pool_ot.tile([128, 512], FP32, tag="ot1", name="ot1"),
                    pool_l.tile([1, 512], FP32, tag="l", name="lp"),
                )
            q0 = qb * 512 + q_off
            ri = ring["i"]; ring["i"] += 1
            s_ps = s_ring[ri % 3]
            for ci in range(2):
                nc.tensor.matmul(
                    s_ps[:, :nqf],
                    kt_sb[:, ci, kt * 128:(kt + 1) * 128],
                    qt_sb[:, ci, q0:q0 + nqf],
                    start=(ci == 0), stop=(ci == 1),
                )
            p_sb = p_ring[ri % 6]
            nc.scalar.activation(
                p_sb[:, :nqf], s_ps[:, :nqf],
                mybir.ActivationFunctionType.Exp, scale=SCALE,
            )
            if kt >= 4 * qb:  # diagonal tile: causal mask on first 128 cols
                nc.vector.tensor_tensor(
                    p_sb[:, :128], p_sb[:, :128], mask_sb[:], mybir.AluOpType.mult
                )
            return p_sb

        def emit_PV(step, p_sb):
            qb, kt, q_off, nqf, first, last = step
            ot0, ot1, lp = state[qb]
            nc.tensor.matmul(ot1[:, q_off:], v_sb[:, kt, 0:128], p_sb[:, :nqf],
                             start=first, stop=last)
            nc.tensor.matmul(ot0[:, q_off:], v_sb[:, kt, 128:256], p_sb[:, :nqf],
                             start=first, stop=last)

        def emit_L(step, p_sb):
            qb, kt, q_off, nqf, first, last = step
            ot0, ot1, lp = state[qb]
            nc.tensor.matmul(lp[:, q_off:], ones_sb[:], p_sb[:, :nqf],
                             start=first, stop=last)
            if last:
                rl_sb = pool_misc.tile([1, 512], FP32, tag="rl", name="rl")
                rb_sb = pool_misc.tile([128, 512], FP32, tag="rb", name="rb")
                nc.vector.reciprocal_approx_fast(rl_sb[:], lp[:])
                nc.gpsimd.partition_broadcast(rb_sb[:], rl_sb[:])
                for ct, otp in ((0, ot1), (1, ot0)):
                    nc.vector.tensor_tensor(
                        ot_sb[:, hp * 2 + ct, qb * 512:(qb + 1) * 512],
                        otp[:], rb_sb[:], mybir.AluOpType.mult,
                    )

        # software pipeline; PE emission order per slot is
        #   S(i), L(i-2), PV(i-1), [next head's QKV block]
        work = []
        emitted = 0
        for i, step in enumerate(steps):
            work.append((step, emit_S(step)))
            if i >= 2:
                emit_L(*work[i - 2])
            if i >= 1:
                emit_PV(*work[i - 1])
            want = (i + 1) * len(next_blocks) // len(steps)
            while emitted < want:
                next_blocks[emitted]()
                emitted += 1
        n = len(steps)
        if n >= 2:
            emit_L(*work[n - 2])
        emit_PV(*work[n - 1])
        emit_L(*work[n - 1])
        while emitted < len(next_blocks):
            next_blocks[emitted]()
            emitted += 1

    head_tiles = alloc_head_tiles()
    for b in qkv_blocks(0, head_tiles):
        b()
    for hp in range(HPC):
        if hp + 1 < HPC:
            nxt_tiles = alloc_head_tiles()
            nxt = qkv_blocks(hp + 1, nxt_tiles)
        else:
            nxt_tiles, nxt = None, []
        attention(hp, head_tiles, nxt)
        head_tiles = nxt_tiles

    # --- output projection, all heads accumulated in PSUM ---
    # round-robin psum across all three tag families (attention pools are
    # done by now) for more outstanding chunks
    prj_pools = [(pool_s, "s"), (pool_ot, "ot0"), (pool_ot, "ot1")]
    for it in range(nt):
        pp, ptag = prj_pools[it % 3]
        ps = pp.tile([128, 512], FP32, tag=ptag, name="ps_prj")
        for t in range(2 * HPC):
            nc.tensor.matmul(
                ps[:, :C],
                ot_sb[:, t, it * 128:(it + 1) * 128],
                wproj_sb[:, t, :],
                start=(t == 0), stop=(t == 2 * HPC - 1),
            )
        osb = pool_osb.tile([128, C], FP32, tag="osb", name="osb")
        nc.vector.tensor_tensor(osb[:], ps[:, :C], biasb_sb[:], mybir.AluOpType.add)
        nc.sync.dma_start(out_r[:, it, :], osb[:])


def build_program(nq=N):
    nc = bacc.Bacc(trn_type="TRN2")
    xt_d = nc.dram_tensor("xt", (C, nq), BF, kind="ExternalInput").ap()
    wqkv_d = nc.dram_tensor("wqkv", (C, 3 * HPC * C), BF, kind="ExternalInput").ap()
    wproj_d = nc.dram_tensor("wproj", (2 * HPC * 128, C), BF, kind="ExternalInput").ap()
    bias_d = nc.dram_tensor("bias", (1, C), FP32, kind="ExternalInput").ap()
    mask_d = nc.dram_tensor("mask", (128, 128), BF, kind="ExternalInput").ap()
    ones_d = nc.dram_tensor("ones", (128, 1), BF, kind="ExternalInput").ap()
    out_d = nc.dram_tensor("out", (nq, C), FP32, kind="ExternalOutput").ap()
    with tile.TileContext(nc) as tc:
        import contextlib
        tc._es = contextlib.ExitStack()
        with tc._es:
            _emit(tc, nq, (xt_d, wqkv_d, wproj_d, bias_d, mask_d, ones_d, out_d))
    nc.compile()
    return nc


def core_inputs(core, x, w_qkv, w_proj, b_proj, nq=N):
    b, hg = core // 2, core % 2
    heads = list(range(hg * HPC, hg * HPC + HPC))
    xt = np.ascontiguousarray(x[b].T).astype(BF16)
    wr = np.asarray(w_qkv, np.float32).reshape(C, 3, H, C)
    w4 = np.ascontiguousarray(
        wr[:, :, heads, :].transpose(0, 2, 1, 3)
    ).reshape(C, 3 * HPC * C).astype(BF16)
    wp = np.asarray(w_proj, np.float32).reshape(H, C, C)[heads].reshape(HPC * C, C).astype(BF16)
    bias = (np.asarray(b_proj, np.float32) if hg == 0
            else np.zeros(C, np.float32)).reshape(1, C)
    mask = (np.arange(128)[:, None] <= np.arange(128)[None, :]).astype(BF16)
    ones = np.ones((128, 1), BF16)
    return {"xt": xt, "wqkv": w4, "wproj": wp, "bias": bias,
            "mask": mask, "ones": ones}


_CACHE = {}


def kernel(x, w_qkv, w_proj, b_proj, **run_kwargs):
    x = np.asarray(x, np.float32)
    w_qkv = np.asarray(w_qkv, np.float32)
    w_proj = np.asarray(w_proj, np.float32)
    b_proj = np.asarray(b_proj, np.float32)
    if "nc" not in _CACHE:
        _CACHE["nc"] = build_program(N)
    nc = _CACHE["nc"]
    in_maps = [core_inputs(c, x, w_qkv, w_proj, b_proj) for c in range(8)]
    res = run_bass_kernel_spmd(nc, in_maps, core_ids=list(range(8)), **run_kwargs)
    out = np.zeros((B, N, C), np.float32)
    for c in range(8):
        out[c // 2] += res.results[c]["out"]
    _CACHE["last_results"] = res
    return out
